# revision 41
# baseline (speedup 1.0000x reference)
"""CapsNet dynamic-routing kernel for TRN2, 8 NeuronCores, data-parallel over batch.

Routing math is fully batch-local; the kernel is a per-batch pipeline hidden
under the u-vec DMA stream:

  host: iter-0 (softmax(0) is uniform) -> outputs0, w20 = W @ outputs0 shipped
  chip: per batch b:  b1 = w20 @ u^T -> softmax -> v1 = c1^T u   (as u lands)
        per group:    pre1 = v1 @ W -> squash -> w21 = W @ out1  (T-pair pipelined)
        per batch:    b2 = w21 @ u^T -> softmax -> v2
        all batches:  pre2 -> one wide squash -> single output DMA

Perf structure (measured on HW traces):
  - the DMA stream of u/u^T (16.8MB fp16) is the floor (~350-425 GB/s); the
    kernel is paced by per-batch arrival, and the tail after the last batch
    is the optimization target.  fp8 was simulated: softmax amplifies
    logit-path quantization to ~2.5e-2 rel err (budget 2e-2) - not usable.
  - u and u^T ship as ONE DMA per batch (same completion gate, fewer
    transfer boundaries); the last two batches ship u^T/u separately so the
    logits matmul can prestart during the final u transfer.
  - squash scale = exp(-0.5*ln(|pre|^2)): Ln and Exp live in the same act
    table set (natural_log_exp_and_others) => ONE act-table load total
    (Sqrt lives in a different set; Exp<->Sqrt swapping cost ~10us).
  - program order == per-engine queue order (in-order queues): every block
    is emitted in input-arrival order; nothing DMA-independent sits in
    front of DMA-dependent work.  iter-2 work packs pairwise (bup(b+1)
    between bup(b) and vmm(b)) so softmax chains hide under PE work.
  - the 4 pre pieces of a group share ONE 2KB PSUM tile (sub-tile range
    dependency tracking verified on HW), so pieces run at PE rate instead
    of serializing on a 2-buffer rotation against the squash chains.
  - iter-2 pre/squash is ONE merged 8-batch block: half the matmuls of two
    4-batch blocks, one wide squash chain, one output DMA.
  - routing-domain capsule columns (b/e/c/vT/w2T) use a permuted order
    pi(n)=8m+2g+tl for n=4(2m+tl)+g, making the w2 PSUM->SBUF gather a
    single 3D strided copy per piece instead of an 8-copy barrier.
  - the masked L copies read the transpose result straight from PSUM (one
    PSUM operand per DVE op is legal; two is not).
fp16 operands / fp32 accumulation.
"""

import numpy as np

ROUTINGS = 3
NC_CAP = 32
DC = 64
EPS = 1e-7
N_CORES = 8
B, N_IN, D_IN = 64, 1024, 512
B_LOC = B // N_CORES  # 8

# routing-domain capsule permutation: capsule n=4(2m+tl)+g lives in column
# pi(n)=8m+2g+tl of b/e/c/vT/w2T (makes the w2 gather a single strided copy)
PI = np.empty(NC_CAP, dtype=np.int64)
for _m in range(4):
    for _tl in range(2):
        for _g in range(4):
            PI[4 * (2 * _m + _tl) + _g] = 8 * _m + 2 * _g + _tl

_cached = {}


def _build_program():
    import concourse.bass as bass
    import concourse.tile as tile
    from concourse import bacc, mybir
    from concourse.hw_specs import get_activation_tables

    f16 = mybir.dt.float16
    f32 = mybir.dt.float32
    ADD = mybir.AluOpType.add
    AX = mybir.AxisListType.X
    AF = mybir.ActivationFunctionType

    nc = bacc.Bacc("TRN2", target_bir_lowering=False, debug=False,
                   num_devices=N_CORES)
    act_tabs = list(get_activation_tables(nc.m.arch).keys())
    LNEXP_SET = act_tabs.index("natural_log_exp_and_others")

    # host-packed, SBUF-native layouts (partition dim first)
    # uu16 per batch: [:, 0:8] = u^T as [128,8,512] (k-major), [:, 8:16] = u
    w16_d = nc.dram_tensor("w16", [128, 4, NC_CAP * DC], f16, kind="ExternalInput").ap()
    wt16_d = nc.dram_tensor("wt16", [128, 16, D_IN], f16, kind="ExternalInput").ap()
    w20t_d = nc.dram_tensor("w20t", [128, 4, B_LOC, NC_CAP], f16, kind="ExternalInput").ap()
    uu_d = nc.dram_tensor("uu16", [B_LOC, 128, 16, D_IN], f16, kind="ExternalInput").ap()
    ident_d = nc.dram_tensor("ident", [128, 128], f16, kind="ExternalInput").ap()
    out_d = nc.dram_tensor("out", [128, 8 * DC], f32, kind="ExternalOutput").ap()

    with tile.TileContext(nc) as tc:
        with (
            tc.tile_pool(name="big", bufs=1) as big,
            tc.tile_pool(name="work", bufs=1) as work,
            tc.tile_pool(name="sbE", bufs=2) as sbE,
            tc.tile_pool(name="sbP", bufs=2) as sbP,
            tc.tile_pool(name="sbO", bufs=4) as sbO,
            tc.tile_pool(name="psB", bufs=2, space="PSUM") as psB,
            tc.tile_pool(name="psV", bufs=1, space="PSUM") as psV,
            tc.tile_pool(name="psPre", bufs=2, space="PSUM") as psPre,
            tc.tile_pool(name="psT", bufs=2, space="PSUM") as psT,
            tc.tile_pool(name="psW2", bufs=1, space="PSUM") as psW2,
        ):
            # UU[:, b, 0:8] = u^T chunks: UT[k%128, j, i] at [:, b, 2j+i//512]
            # UU[:, b, 8:16] = u chunks:  U[i%128, t, k]  at [:, b, 8+t]
            UU = big.tile([128, B_LOC, 16, D_IN], f16, tag="UU")
            W16 = big.tile([128, 4, NC_CAP * DC], f16, tag="W16")  # (k%128),(k//128),(n d)
            WT16 = big.tile([128, 16, D_IN], f16, tag="WT16")      # (tau d),(m g),(k)
            W20T = big.tile([128, 4, B_LOC, NC_CAP], f16, tag="W20T")
            IDENT = work.tile([128, 128], f16, tag="IDENT")
            JUNK = work.tile([128, 256], f16, tag="JUNK")

            def UTv(b, j, i0):  # UT[:, b, j, i0:i0+128]
                return UU[:, b, 2 * j + i0 // 512,
                          (i0 % 512):(i0 % 512) + 128]

            def Uv(b, t, k0):   # U[:, b, t, k0:k0+128]
                return UU[:, b, 8 + t, k0:k0 + 128]

            # per-group iter-1 tiles (A: batches 0-3, B: 4-7); iter-2 shares
            vT = [work.tile([128, 4, 4, NC_CAP], f16, tag=f"vT{g}",
                            name=f"vT{g}") for g in range(2)]
            vT2 = work.tile([128, 4, B_LOC, NC_CAP], f16, tag="vT2")
            w2T = [work.tile([128, 4, 4, NC_CAP], f16, tag=f"w2T{g}",
                             name=f"w2T{g}") for g in range(2)]
            c_sb = [work.tile([128, 4, 8, NC_CAP], f16, tag=f"c{g}",
                              name=f"c{g}") for g in range(2)]
            L_sb = [[work.tile([128, 4, 2, 4], f16, tag=f"L{g}_{m}",
                               name=f"L{g}_{m}") for m in range(4)]
                    for g in range(2)]
            outp32 = work.tile([128, 8, DC], f32, tag="outp32")
            z_sb = work.tile([128, B_LOC, 8], f32, tag="z")
            r_sb = work.tile([128, B_LOC, 8], f32, tag="r")

            # ---- single act-table load: Ln+Exp (+Copy) in one set ----
            nc.scalar.add_instruction(mybir.InstLoadActFuncSet(
                name=f"I-{nc.next_id()}", act_func_set_id=LNEXP_SET))
            nc.gpsimd.memset(JUNK[:], 0.5)
            for g in range(2):
                for m in range(4):
                    nc.gpsimd.memset(L_sb[g][m][:], 0.0)

            # ---- PE HAM warm-up: ~5us of junk matmuls from t~1.5us ----
            # (shares the psW2 "w2pn" slot: w2 only starts ~40us in)
            dummy_ps = psW2.tile([128, 4, 16, 2, 4], f32, tag="w2pn")
            with nc.named_scope("ham_warmup"):
                for i in range(48):
                    nc.tensor.matmul(dummy_ps[:, i % 4], JUNK[:, 0:128],
                                     JUNK[:, 0:128],
                                     start=True, stop=True)

            def keep_warm(w):
                # junk weight-load reading a just-landed DMA slice: fires on
                # arrival, bridging a PE-idle window (pure LDWEIGHTS: no
                # PSUM write, no WAR with live accumulators)
                nc.tensor.ldweights(w)

            # ---- DMA queue: order == consumption order of the pipeline.
            # Transfers are kept at <=8KB per partition row (16KB rows
            # measured ~430ns inter-packet gaps per engine). Triggers are
            # split across the two HWDGE trigger engines (sync + scalar):
            # each trigger ring allows ~8 transfers in flight and refill
            # costs ~0.9us of dead time per transfer; two engines halve it.
            nc.sync.dma_start(IDENT[:], ident_d[:])
            nc.sync.dma_start(W20T[:], w20t_d[:])
            for b in range(0, 4):
                nc.sync.dma_start(UU[:, b, 0:8], uu_d[b][:, 0:8])
                nc.scalar.dma_start(UU[:, b, 8:16], uu_d[b][:, 8:16])
            nc.scalar.dma_start(W16[:, 0:2], w16_d[:, 0:2])
            nc.sync.dma_start(W16[:, 2:4], w16_d[:, 2:4])
            nc.scalar.dma_start(WT16[:, 0:8], wt16_d[:, 0:8])
            nc.sync.dma_start(WT16[:, 8:16], wt16_d[:, 8:16])
            for b in range(4, B_LOC):
                nc.sync.dma_start(UU[:, b, 0:8], uu_d[b][:, 0:8])
                nc.scalar.dma_start(UU[:, b, 8:16], uu_d[b][:, 8:16])

            def bup(b, it):
                # b-logits for batch b: [i%128, t, n] = sum_k u^T chunks @ w2T
                src = W20T if it == 1 else w2T[b // 4]
                bl = b if it == 1 else b % 4
                with nc.named_scope(f"i{it}_bup{b}"):
                    b_ps = psB.tile([128, 8, NC_CAP], f32, tag="b_ps")
                    for t in range(8):
                        for j in range(4):
                            nc.tensor.matmul(
                                b_ps[:, t], UTv(b, j, 128 * t),
                                src[:, j, bl, :], start=(j == 0), stop=(j == 3))
                    e_sb = sbE.tile([128, 8, NC_CAP], f16, tag="e_sb")
                    nc.scalar.activation(e_sb[:], b_ps[:], AF.Exp)
                    nc.vector.tensor_reduce(z_sb[:, b], e_sb[:], AX, ADD)
                    nc.vector.reciprocal(r_sb[:, b], z_sb[:, b])
                    nc.vector.tensor_mul(
                        c_sb[b // 4][:, b % 4], e_sb[:],
                        r_sb[:, b].broadcast_to((128, 8, NC_CAP)))

            def vmm(b, it):
                with nc.named_scope(f"i{it}_v{b}"):
                    vT_ps = psV.tile([128, 4, NC_CAP], f32, tag="vT_ps")
                    for j in range(4):
                        for t in range(8):
                            nc.tensor.matmul(
                                vT_ps[:, j], Uv(b, t, 128 * j),
                                c_sb[b // 4][:, b % 4, t, :],
                                start=(t == 0), stop=(t == 7))
                    dst = (vT[b // 4][:, :, b % 4, :] if it == 1
                           else vT2[:, :, b, :])
                    nc.scalar.copy(dst, vT_ps[:])

            def pre_halves():
                # two 1KB PSUM tiles per group (pieces 0-1 / 2-3) in
                # DIFFERENT banks: the squash front of half X runs on DVE
                # while half Y's matmuls stream (reading a PSUM bank that
                # the PE still writes serializes at bank granularity)
                pgX = psPre.tile([128, 4, DC], f32, tag="pre", name="pgX")
                pgY = psPre.tile([128, 4, DC], f32, tag="pre", name="pgY")
                return pgX, pgY

            def pre_piece(src, nb, pgh, m):
                # capsules n = 4T+g for T in {2m, 2m+1} -> pgh[:, (2m+tl)%4]
                for tl in range(2):
                    for g in range(4):
                        n = 4 * (2 * m + tl) + g
                        q = 8 * m + 2 * g + tl  # pi(n): vT column order
                        for j in range(4):
                            nc.tensor.matmul(
                                pgh[32 * g:32 * g + nb, (2 * m + tl) % 4],
                                src[:, j, :, q],
                                W16[:, j, 64 * n:64 * n + 64],
                                start=(j == 0), stop=(j == 3),
                                tile_position=(0, 32 * g),
                            )

            def squash_front(pgh, nrm, c0):
                # |pre|^2 for one half: runs while the other half's matmuls
                # stream on the PE. 1/sqrt later as exp(-0.5*ln(x)) (one act
                # table set); |pre|^2 >> 1e-7 so the reference eps is inert
                pre_c = sbP.tile([128, 4, DC], f32, tag="pre_c")
                sq2 = sbP.tile([128, 4, DC], f32, tag="sq2")
                nc.vector.tensor_copy(pre_c[:], pgh[:])
                nc.vector.tensor_mul(sq2[:], pre_c[:], pgh[:])
                nc.vector.tensor_reduce(nrm[:, c0:c0 + 4], sq2[:], AX, ADD)

            def squash_scale(nrm):
                lgn = sbP.tile([128, 8], f32, tag="lgn")
                scl = sbP.tile([128, 8], f32, tag="scl")
                nc.scalar.activation(lgn[:], nrm[:], AF.Ln)
                nc.scalar.activation(scl[:], lgn[:], AF.Exp, scale=-0.5)
                return scl

            def squash_dst(pgh, scl, c0, dst, dsl):
                nc.vector.tensor_mul(
                    dst[:, dsl], pgh[:],
                    scl[:, c0:c0 + 4].broadcast_to((128, 4, DC)))

            def w2_piece(grp, w2pn, m, o16):
                # transpose scaled outputs T-pair m -> tp[(tau d), (g c)],
                # mask into L straight from PSUM, contract d, then gather
                # with ONE strided copy (pi() makes (g,tau) a stride-4 run)
                tp = psT.tile([128, 128], f16, tag="tp")
                nc.tensor.transpose(
                    tp[:], o16[:, 2 * m:2 * m + 2].rearrange("p a b -> p (a b)"),
                    IDENT[:])
                for tau in range(2):
                    nc.vector.tensor_copy(
                        L_sb[grp][m][64 * tau:64 * tau + 64, :, tau, :],
                        tp[64 * tau:64 * tau + 64, :]
                        .rearrange("p (g c) -> p g c", g=4)[:, :, 0:4])
                for p in range(4 * m, 4 * m + 4):
                    for j in range(4):
                        nc.tensor.matmul(
                            w2pn[:, j, p], WT16[:, p, 128 * j:128 * j + 128],
                            L_sb[grp][m][:, p - 4 * m], start=True, stop=True)
                nc.scalar.copy(
                    w2T[grp][:, :, :, 8 * m:8 * m + 8],
                    w2pn[:, :, 4 * m:4 * m + 4, :, :].rearrange(
                        "p j g x b -> p j b (g x)"))

            # ================= schedule =================
            # program order == per-engine queue order; blocks emitted in the
            # order their inputs land (in-order queues => head-of-line)
            bup(0, 1); vmm(0, 1)
            bup(1, 1); vmm(1, 1)
            bup(2, 1); vmm(2, 1)
            bup(3, 1); vmm(3, 1)
            def pre1_squash_w2(grp, w2pn, mid=None):
                # `mid` (optional thunk) emits PE work between the first and
                # second pair of w2 pieces -- used to slot an arriving
                # batch's logits in so it isn't queue-blocked by the block
                pgX, pgY = pre_halves()
                nrm = sbP.tile([128, 8], f32, tag="nrm", name="nrm")
                o = sbO.tile([128, 8, DC], f16, tag="o16", name="o16")
                pre_piece(vT[grp], 4, pgX, 0)
                pre_piece(vT[grp], 4, pgX, 1)
                pre_piece(vT[grp], 4, pgY, 2)
                pre_piece(vT[grp], 4, pgY, 3)
                squash_front(pgX, nrm, 0)   # DVE, overlaps pgY matmuls
                squash_front(pgY, nrm, 4)
                scl = squash_scale(nrm)
                squash_dst(pgX, scl, 0, o, slice(0, 4))
                w2_piece(grp, w2pn, 0, o)
                w2_piece(grp, w2pn, 1, o)
                squash_dst(pgY, scl, 4, o, slice(4, 8))
                if mid is not None:
                    mid()
                w2_piece(grp, w2pn, 2, o)
                w2_piece(grp, w2pn, 3, o)

            w2pnA = psW2.tile([128, 4, 16, 2, 4], f32, tag="w2pn")
            keep_warm(W16[:, 0, 0:128])
            pre1_squash_w2(0, w2pnA, mid=lambda: bup(4, 1))
            keep_warm(WT16[:, 0, 0:128])
            # group-B iter-1 as batches land, iter-2 A packed into the
            # gaps; the final bups are pulled forward so the tail-gating
            # batch-7 logits run the moment their data lands
            bup(0, 2)
            vmm(4, 1); vmm(0, 2)
            bup(5, 1); bup(1, 2)
            vmm(5, 1); vmm(1, 2)
            bup(6, 1); bup(2, 2)
            bup(7, 1)
            vmm(6, 1); vmm(2, 2)
            bup(3, 2)
            vmm(7, 1); vmm(3, 2)
            # ---- tail: everything below is gated by the last batch ----
            w2pnB = psW2.tile([128, 4, 16, 2, 4], f32, tag="w2pn")
            pre1_squash_w2(1, w2pnB)
            # iter-2 B packed pairwise: bup(b+1) runs while softmax(b) chases
            bup(4, 2); bup(5, 2)
            vmm(4, 2); bup(6, 2)
            vmm(5, 2); bup(7, 2)
            vmm(6, 2); vmm(7, 2)
            # merged 8-batch pre2; output DMA fires per squash half
            pg2X, pg2Y = pre_halves()
            nrm2 = sbP.tile([128, 8], f32, tag="nrm", name="nrm2")
            pre_piece(vT2, 8, pg2X, 0)
            pre_piece(vT2, 8, pg2X, 1)
            pre_piece(vT2, 8, pg2Y, 2)
            pre_piece(vT2, 8, pg2Y, 3)
            squash_front(pg2X, nrm2, 0)
            squash_front(pg2Y, nrm2, 4)
            scl2 = squash_scale(nrm2)
            squash_dst(pg2X, scl2, 0, outp32, slice(0, 4))
            nc.sync.dma_start(out_d[:, 0:256],
                              outp32[:, 0:4].rearrange("p a b -> p (a b)"))
            squash_dst(pg2Y, scl2, 4, outp32, slice(4, 8))
            nc.sync.dma_start(out_d[:, 256:512],
                              outp32[:, 4:8].rearrange("p a b -> p (a b)"))

    nc.compile()
    return nc


def _host_prep(u_vecs, W):
    u_vecs = np.asarray(u_vecs, dtype=np.float32)
    W = np.asarray(W, dtype=np.float32).reshape(D_IN, NC_CAP * DC)
    Wr = W.reshape(D_IN, NC_CAP, DC)

    w16 = np.ascontiguousarray(
        W.reshape(4, 128, NC_CAP * DC).transpose(1, 0, 2)).astype(np.float16)
    # WT packed: [128=(tau,d), 16=(m,g), 512]; capsule n = 8m + 4tau + g
    wt = np.zeros((128, 16, D_IN), dtype=np.float16)
    for m in range(4):
        for g in range(4):
            for tau in range(2):
                n = 8 * m + 4 * tau + g
                wt[64 * tau:64 * tau + 64, 4 * m + g, :] = \
                    Wr[:, n, :].T.astype(np.float16)
    ident = np.eye(128, dtype=np.float16)

    in_maps = []
    for c in range(N_CORES):
        ub = u_vecs[c * B_LOC:(c + 1) * B_LOC]  # [8, 1024, 512] fp32
        u16 = ub.astype(np.float16)
        up = np.ascontiguousarray(
            u16.reshape(B_LOC, 8, 128, D_IN).transpose(0, 2, 1, 3))
        utp = np.ascontiguousarray(
            u16.transpose(0, 2, 1).reshape(B_LOC, 4, 128, N_IN)
            .transpose(0, 2, 1, 3))
        uu = np.ascontiguousarray(np.concatenate(
            [utp.reshape(B_LOC, 128, 8, D_IN), up], axis=2))
        # host iter-0: c is uniform, so outputs0 depends only on column sums
        s = ub.sum(axis=1) / NC_CAP                       # [8, 512] fp32
        pre0 = np.einsum('bk,knd->bnd', s, Wr)
        out0 = pre0 / np.sqrt((pre0 ** 2).sum(-1, keepdims=True) + EPS)
        w20 = np.einsum('bnd,knd->bnk', out0, Wr)         # [8, 32, 512]
        w20 = w20[:, np.argsort(PI), :]  # routing-domain column order pi(n)
        w20t = np.ascontiguousarray(
            w20.transpose(2, 0, 1).reshape(4, 128, B_LOC, NC_CAP)
            .transpose(1, 0, 2, 3)).astype(np.float16)
        in_maps.append({
            "uu16": uu, "w16": w16, "wt16": wt, "w20t": w20t,
            "ident": ident,
        })
    return in_maps


def _unpack_out(raw):
    # raw [128, 512] f32; row 32g+q, cols (T, d) -> out[q, 4T+g, d]
    out = np.empty((B_LOC, NC_CAP, DC), dtype=np.float32)
    r = raw.reshape(4, 32, 8, DC)   # [g, q-slot, T, d]
    for g in range(4):
        for q in range(B_LOC):
            out[q, 4 * np.arange(8) + g, :] = r[g, q]
    return out


def kernel(u_vecs, W):
    from concourse.bass_utils import run_bass_kernel_spmd

    if "nc" not in _cached:
        _cached["nc"] = _build_program()
    nc = _cached["nc"]

    in_maps = _host_prep(u_vecs, W)
    res = run_bass_kernel_spmd(nc, in_maps, list(range(N_CORES)))
    out = np.concatenate(
        [_unpack_out(res.results[c]["out"]) for c in range(N_CORES)], axis=0)
    return out.astype(np.float32)


# revision 47
# speedup vs baseline: 1.1112x; 1.1112x over previous
"""CapsNet dynamic-routing kernel for TRN2, 8 NeuronCores, data-parallel over batch.

Routing math is fully batch-local; the kernel is a per-batch pipeline hidden
under the u-vec DMA stream:

  host: iter-0 (softmax(0) is uniform) -> outputs0, w20 = W @ outputs0 shipped
  chip: per batch b:  b1 = w20 @ u^T -> softmax -> v1 = c1^T u   (as u lands)
        per group:    pre1 = v1 @ W -> squash -> w21 = W @ out1  (T-pair pipelined)
        per batch:    b2 = w21 @ u^T -> softmax -> v2
        all batches:  pre2 -> one wide squash -> single output DMA

Perf structure (measured on HW traces):
  - the DMA stream of u/u^T (16.8MB fp16) is the floor (~350-425 GB/s); the
    kernel is paced by per-batch arrival, and the tail after the last batch
    is the optimization target.  fp8 was simulated: softmax amplifies
    logit-path quantization to ~2.5e-2 rel err (budget 2e-2) - not usable.
  - u and u^T ship as ONE DMA per batch (same completion gate, fewer
    transfer boundaries); the last two batches ship u^T/u separately so the
    logits matmul can prestart during the final u transfer.
  - squash scale = exp(-0.5*ln(|pre|^2)): Ln and Exp live in the same act
    table set (natural_log_exp_and_others) => ONE act-table load total
    (Sqrt lives in a different set; Exp<->Sqrt swapping cost ~10us).
  - program order == per-engine queue order (in-order queues): every block
    is emitted in input-arrival order; nothing DMA-independent sits in
    front of DMA-dependent work.  iter-2 work packs pairwise (bup(b+1)
    between bup(b) and vmm(b)) so softmax chains hide under PE work.
  - the 4 pre pieces of a group share ONE 2KB PSUM tile (sub-tile range
    dependency tracking verified on HW), so pieces run at PE rate instead
    of serializing on a 2-buffer rotation against the squash chains.
  - iter-2 pre/squash is ONE merged 8-batch block: half the matmuls of two
    4-batch blocks, one wide squash chain, one output DMA.
  - routing-domain capsule columns (b/e/c/vT/w2T) use a permuted order
    pi(n)=8m+2g+tl for n=4(2m+tl)+g, making the w2 PSUM->SBUF gather a
    single 3D strided copy per piece instead of an 8-copy barrier.
  - the masked L copies read the transpose result straight from PSUM (one
    PSUM operand per DVE op is legal; two is not).
fp16 operands / fp32 accumulation.
"""

import numpy as np

ROUTINGS = 3
NC_CAP = 32
DC = 64
EPS = 1e-7
N_CORES = 8
B, N_IN, D_IN = 64, 1024, 512
B_LOC = B // N_CORES  # 8

# routing-domain capsule permutation: capsule n=4(2m+tl)+g lives in column
# pi(n)=8m+2g+tl of b/e/c/vT/w2T (makes the w2 gather a single strided copy)
PI = np.empty(NC_CAP, dtype=np.int64)
for _m in range(4):
    for _tl in range(2):
        for _g in range(4):
            PI[4 * (2 * _m + _tl) + _g] = 8 * _m + 2 * _g + _tl

_cached = {}


def _build_program():
    import concourse.bass as bass
    import concourse.tile as tile
    from concourse import bacc, mybir
    from concourse.hw_specs import get_activation_tables

    f16 = mybir.dt.float16
    f32 = mybir.dt.float32
    ADD = mybir.AluOpType.add
    AX = mybir.AxisListType.X
    AF = mybir.ActivationFunctionType

    nc = bacc.Bacc("TRN2", target_bir_lowering=False, debug=False,
                   num_devices=N_CORES)
    act_tabs = list(get_activation_tables(nc.m.arch).keys())
    LNEXP_SET = act_tabs.index("natural_log_exp_and_others")

    # host-packed, SBUF-native layouts (partition dim first)
    # uu16 per batch: [:, 0:8] = u^T as [128,8,512] (k-major), [:, 8:16] = u
    w16_d = nc.dram_tensor("w16", [128, 4, NC_CAP * DC], f16, kind="ExternalInput").ap()
    wt16_d = nc.dram_tensor("wt16", [128, 16, D_IN], f16, kind="ExternalInput").ap()
    w20t_d = nc.dram_tensor("w20t", [128, 4, B_LOC, NC_CAP], f16, kind="ExternalInput").ap()
    uu_d = nc.dram_tensor("uu16", [B_LOC, 128, 16, D_IN], f16, kind="ExternalInput").ap()
    ident_d = nc.dram_tensor("ident", [128, 128], f16, kind="ExternalInput").ap()
    out_d = nc.dram_tensor("out", [128, 8 * DC], f32, kind="ExternalOutput").ap()

    with tile.TileContext(nc) as tc:
        with (
            tc.tile_pool(name="big", bufs=1) as big,
            tc.tile_pool(name="work", bufs=1) as work,
            tc.tile_pool(name="sbE", bufs=2) as sbE,
            tc.tile_pool(name="sbP", bufs=2) as sbP,
            tc.tile_pool(name="sbO", bufs=4) as sbO,
            tc.tile_pool(name="psB", bufs=2, space="PSUM") as psB,
            tc.tile_pool(name="psV", bufs=1, space="PSUM") as psV,
            tc.tile_pool(name="psPre", bufs=2, space="PSUM") as psPre,
            tc.tile_pool(name="psT", bufs=2, space="PSUM") as psT,
            tc.tile_pool(name="psW2", bufs=1, space="PSUM") as psW2,
        ):
            # UU[:, b, 0:8] = u^T chunks: UT[k%128, j, i] at [:, b, 2j+i//512]
            # UU[:, b, 8:16] = u chunks:  U[i%128, t, k]  at [:, b, 8+t]
            UU = big.tile([128, B_LOC, 16, D_IN], f16, tag="UU")
            W16 = big.tile([128, 4, NC_CAP * DC], f16, tag="W16")  # (k%128),(k//128),(n d)
            WT16 = big.tile([128, 16, D_IN], f16, tag="WT16")      # (tau d),(m g),(k)
            W20T = big.tile([128, 4, B_LOC, NC_CAP], f16, tag="W20T")
            IDENT = work.tile([128, 128], f16, tag="IDENT")
            JUNK = work.tile([128, 256], f16, tag="JUNK")

            def UTv(b, j, i0):  # UT[:, b, j, i0:i0+128]
                return UU[:, b, 2 * j + i0 // 512,
                          (i0 % 512):(i0 % 512) + 128]

            def Uv(b, t, k0):   # U[:, b, t, k0:k0+128]
                return UU[:, b, 8 + t, k0:k0 + 128]

            # per-group iter-1 tiles (A: batches 0-3, B: 4-7); iter-2 shares
            vT = [work.tile([128, 4, 4, NC_CAP], f16, tag=f"vT{g}",
                            name=f"vT{g}") for g in range(2)]
            vT2 = work.tile([128, 4, B_LOC, NC_CAP], f16, tag="vT2")
            w2T = [work.tile([128, 4, 4, NC_CAP], f16, tag=f"w2T{g}",
                             name=f"w2T{g}") for g in range(2)]
            c_sb = [work.tile([128, 4, 8, NC_CAP], f16, tag=f"c{g}",
                              name=f"c{g}") for g in range(2)]
            L_sb = [[work.tile([128, 4, 2, 4], f16, tag=f"L{g}_{m}",
                               name=f"L{g}_{m}") for m in range(4)]
                    for g in range(2)]
            outp32 = work.tile([128, 8, DC], f32, tag="outp32")
            z_sb = work.tile([128, B_LOC, 8], f32, tag="z")
            r_sb = work.tile([128, B_LOC, 8], f32, tag="r")

            # ---- single act-table load: Ln+Exp (+Copy) in one set ----
            nc.scalar.add_instruction(mybir.InstLoadActFuncSet(
                name=f"I-{nc.next_id()}", act_func_set_id=LNEXP_SET))
            nc.gpsimd.memset(JUNK[:], 0.5)
            for g in range(2):
                for m in range(4):
                    nc.gpsimd.memset(L_sb[g][m][:], 0.0)

            # ---- PE HAM warm-up: ~5us of junk matmuls from t~1.5us ----
            # (shares the psW2 "w2pn" slot: w2 only starts ~40us in)
            dummy_ps = psW2.tile([128, 4, 16, 2, 4], f32, tag="w2pn")
            with nc.named_scope("ham_warmup"):
                for i in range(48):
                    nc.tensor.matmul(dummy_ps[:, i % 4], JUNK[:, 0:128],
                                     JUNK[:, 0:128],
                                     start=True, stop=True)

            def keep_warm(w):
                # junk weight-load reading a just-landed DMA slice: fires on
                # arrival, bridging a PE-idle window (pure LDWEIGHTS: no
                # PSUM write, no WAR with live accumulators)
                nc.tensor.ldweights(w)

            # ---- DMA queue: order == consumption order of the pipeline.
            # Transfers are kept at <=8KB per partition row (16KB rows
            # measured ~430ns inter-packet gaps per engine). Triggers are
            # split across the two HWDGE trigger engines (sync + scalar):
            # each trigger ring allows ~8 transfers in flight and refill
            # costs ~0.9us of dead time per transfer; two engines halve it.
            nc.sync.dma_start(IDENT[:], ident_d[:])
            nc.sync.dma_start(W20T[:], w20t_d[:])
            for b in range(0, 2):
                nc.sync.dma_start(UU[:, b, 0:8], uu_d[b][:, 0:8])
                nc.sync.dma_start(UU[:, b, 8:16], uu_d[b][:, 8:16])
            nc.sync.dma_start(W16[:, 0:2], w16_d[:, 0:2])
            nc.sync.dma_start(W16[:, 2:4], w16_d[:, 2:4])
            for b in range(2, 4):
                nc.sync.dma_start(UU[:, b, 0:8], uu_d[b][:, 0:8])
                nc.sync.dma_start(UU[:, b, 8:16], uu_d[b][:, 8:16])
            nc.sync.dma_start(WT16[:, 0:8], wt16_d[:, 0:8])
            nc.sync.dma_start(WT16[:, 8:16], wt16_d[:, 8:16])
            for b in range(4, 7):
                nc.sync.dma_start(UU[:, b, 0:8], uu_d[b][:, 0:8])
                nc.sync.dma_start(UU[:, b, 8:16], uu_d[b][:, 8:16])
            # the tail-gating batch ships in quarters so its logits/v
            # matmuls consume partial data as it lands (loops below
            # accumulate in matching order)
            nc.sync.dma_start(UU[:, 7, 0:4], uu_d[7][:, 0:4])
            nc.sync.dma_start(UU[:, 7, 4:8], uu_d[7][:, 4:8])
            nc.sync.dma_start(UU[:, 7, 8:12], uu_d[7][:, 8:12])
            nc.sync.dma_start(UU[:, 7, 12:16], uu_d[7][:, 12:16])

            def bup(b, it):
                # b-logits for batch b: [i%128, t, n] = sum_k u^T chunks @ w2T
                src = W20T if it == 1 else w2T[b // 4]
                bl = b if it == 1 else b % 4
                with nc.named_scope(f"i{it}_bup{b}"):
                    b_ps = psB.tile([128, 8, NC_CAP], f32, tag="b_ps")
                    for t in range(8):
                        for j in range(4):
                            nc.tensor.matmul(
                                b_ps[:, t], UTv(b, j, 128 * t),
                                src[:, j, bl, :], start=(j == 0), stop=(j == 3))
                    e_sb = sbE.tile([128, 8, NC_CAP], f16, tag="e_sb")
                    nc.scalar.activation(e_sb[:], b_ps[:], AF.Exp)
                    nc.vector.tensor_reduce(z_sb[:, b], e_sb[:], AX, ADD)
                    nc.vector.reciprocal(r_sb[:, b], z_sb[:, b])
                    nc.vector.tensor_mul(
                        c_sb[b // 4][:, b % 4], e_sb[:],
                        r_sb[:, b].broadcast_to((128, 8, NC_CAP)))

            def vmm(b, it):
                with nc.named_scope(f"i{it}_v{b}"):
                    vT_ps = psV.tile([128, 4, NC_CAP], f32, tag="vT_ps")
                    for j in range(4):
                        for t in range(8):
                            nc.tensor.matmul(
                                vT_ps[:, j], Uv(b, t, 128 * j),
                                c_sb[b // 4][:, b % 4, t, :],
                                start=(t == 0), stop=(t == 7))
                    dst = (vT[b // 4][:, :, b % 4, :] if it == 1
                           else vT2[:, :, b, :])
                    nc.scalar.copy(dst, vT_ps[:])

            def pre_halves():
                # two 1KB PSUM tiles per group (pieces 0-1 / 2-3) in
                # DIFFERENT banks: the squash front of half X runs on DVE
                # while half Y's matmuls stream (reading a PSUM bank that
                # the PE still writes serializes at bank granularity)
                pgX = psPre.tile([128, 4, DC], f32, tag="pre", name="pgX")
                pgY = psPre.tile([128, 4, DC], f32, tag="pre", name="pgY")
                return pgX, pgY

            def pre_piece(src, nb, pgh, m):
                # capsules n = 4T+g for T in {2m, 2m+1} -> pgh[:, (2m+tl)%4]
                for tl in range(2):
                    for g in range(4):
                        n = 4 * (2 * m + tl) + g
                        q = 8 * m + 2 * g + tl  # pi(n): vT column order
                        for j in range(4):
                            nc.tensor.matmul(
                                pgh[32 * g:32 * g + nb, (2 * m + tl) % 4],
                                src[:, j, :, q],
                                W16[:, j, 64 * n:64 * n + 64],
                                start=(j == 0), stop=(j == 3),
                                tile_position=(0, 32 * g),
                            )

            def squash_front(pgh, nrm, c0):
                # |pre|^2 for one half: runs while the other half's matmuls
                # stream on the PE. 1/sqrt later as exp(-0.5*ln(x)) (one act
                # table set); |pre|^2 >> 1e-7 so the reference eps is inert
                pre_c = sbP.tile([128, 4, DC], f32, tag="pre_c")
                sq2 = sbP.tile([128, 4, DC], f32, tag="sq2")
                nc.vector.tensor_copy(pre_c[:], pgh[:])
                nc.vector.tensor_mul(sq2[:], pre_c[:], pgh[:])
                nc.vector.tensor_reduce(nrm[:, c0:c0 + 4], sq2[:], AX, ADD)

            def squash_scale(nrm):
                lgn = sbP.tile([128, 8], f32, tag="lgn")
                scl = sbP.tile([128, 8], f32, tag="scl")
                nc.scalar.activation(lgn[:], nrm[:], AF.Ln)
                nc.scalar.activation(scl[:], lgn[:], AF.Exp, scale=-0.5)
                return scl

            def squash_dst(pgh, scl, c0, dst, dsl):
                nc.vector.tensor_mul(
                    dst[:, dsl], pgh[:],
                    scl[:, c0:c0 + 4].broadcast_to((128, 4, DC)))

            def w2_piece(grp, w2pn, m, o16):
                # transpose scaled outputs T-pair m -> tp[(tau d), (g c)],
                # mask into L straight from PSUM, contract d, then gather
                # with ONE strided copy (pi() makes (g,tau) a stride-4 run)
                tp = psT.tile([128, 128], f16, tag="tp")
                nc.tensor.transpose(
                    tp[:], o16[:, 2 * m:2 * m + 2].rearrange("p a b -> p (a b)"),
                    IDENT[:])
                for tau in range(2):
                    nc.vector.tensor_copy(
                        L_sb[grp][m][64 * tau:64 * tau + 64, :, tau, :],
                        tp[64 * tau:64 * tau + 64, :]
                        .rearrange("p (g c) -> p g c", g=4)[:, :, 0:4])
                for p in range(4 * m, 4 * m + 4):
                    for j in range(4):
                        nc.tensor.matmul(
                            w2pn[:, j, p], WT16[:, p, 128 * j:128 * j + 128],
                            L_sb[grp][m][:, p - 4 * m], start=True, stop=True)
                nc.scalar.copy(
                    w2T[grp][:, :, :, 8 * m:8 * m + 8],
                    w2pn[:, :, 4 * m:4 * m + 4, :, :].rearrange(
                        "p j g x b -> p j b (g x)"))

            # ================= schedule =================
            # program order == per-engine queue order; blocks emitted in the
            # order their inputs land (in-order queues => head-of-line)
            bup(0, 1); vmm(0, 1)
            bup(1, 1); vmm(1, 1)
            bup(2, 1); vmm(2, 1)
            bup(3, 1); vmm(3, 1)
            def pre1_squash_w2(grp, w2pn, mid=None):
                # `mid` (optional thunk) emits PE work between the first and
                # second pair of w2 pieces -- used to slot an arriving
                # batch's logits in so it isn't queue-blocked by the block
                pgX, pgY = pre_halves()
                nrm = sbP.tile([128, 8], f32, tag="nrm", name="nrm")
                o = sbO.tile([128, 8, DC], f16, tag="o16", name="o16")
                pre_piece(vT[grp], 4, pgX, 0)
                pre_piece(vT[grp], 4, pgX, 1)
                pre_piece(vT[grp], 4, pgY, 2)
                pre_piece(vT[grp], 4, pgY, 3)
                squash_front(pgX, nrm, 0)   # DVE, overlaps pgY matmuls
                squash_front(pgY, nrm, 4)
                scl = squash_scale(nrm)
                squash_dst(pgX, scl, 0, o, slice(0, 4))
                w2_piece(grp, w2pn, 0, o)
                w2_piece(grp, w2pn, 1, o)
                squash_dst(pgY, scl, 4, o, slice(4, 8))
                if mid is not None:
                    mid()
                w2_piece(grp, w2pn, 2, o)
                w2_piece(grp, w2pn, 3, o)

            w2pnA = psW2.tile([128, 4, 16, 2, 4], f32, tag="w2pn")
            keep_warm(W16[:, 0, 0:128])
            pre1_squash_w2(0, w2pnA, mid=lambda: bup(4, 1))
            keep_warm(WT16[:, 0, 0:128])
            # group-B iter-1 as batches land, iter-2 A packed into the
            # gaps; the final bups are pulled forward so the tail-gating
            # batch-7 logits run the moment their data lands
            bup(0, 2)
            vmm(4, 1); vmm(0, 2)
            bup(5, 1); bup(1, 2)
            vmm(5, 1); vmm(1, 2)
            bup(6, 1); bup(2, 2)
            bup(7, 1)
            vmm(6, 1); vmm(2, 2)
            bup(3, 2)
            vmm(7, 1); vmm(3, 2)
            # ---- tail: everything below is gated by the last batch ----
            w2pnB = psW2.tile([128, 4, 16, 2, 4], f32, tag="w2pn")
            pre1_squash_w2(1, w2pnB)
            # iter-2 B packed pairwise: bup(b+1) runs while softmax(b) chases
            bup(4, 2); bup(5, 2)
            vmm(4, 2); bup(6, 2)
            vmm(5, 2); bup(7, 2)
            vmm(6, 2); vmm(7, 2)
            # merged 8-batch pre2; output DMA fires per squash half
            pg2X, pg2Y = pre_halves()
            nrm2 = sbP.tile([128, 8], f32, tag="nrm", name="nrm2")
            pre_piece(vT2, 8, pg2X, 0)
            pre_piece(vT2, 8, pg2X, 1)
            pre_piece(vT2, 8, pg2Y, 2)
            pre_piece(vT2, 8, pg2Y, 3)
            squash_front(pg2X, nrm2, 0)
            squash_front(pg2Y, nrm2, 4)
            scl2 = squash_scale(nrm2)
            squash_dst(pg2X, scl2, 0, outp32, slice(0, 4))
            nc.sync.dma_start(out_d[:, 0:256],
                              outp32[:, 0:4].rearrange("p a b -> p (a b)"))
            squash_dst(pg2Y, scl2, 4, outp32, slice(4, 8))
            nc.sync.dma_start(out_d[:, 256:512],
                              outp32[:, 4:8].rearrange("p a b -> p (a b)"))

    nc.compile()
    return nc


def _host_prep(u_vecs, W):
    u_vecs = np.asarray(u_vecs, dtype=np.float32)
    W = np.asarray(W, dtype=np.float32).reshape(D_IN, NC_CAP * DC)
    Wr = W.reshape(D_IN, NC_CAP, DC)

    w16 = np.ascontiguousarray(
        W.reshape(4, 128, NC_CAP * DC).transpose(1, 0, 2)).astype(np.float16)
    # WT packed: [128=(tau,d), 16=(m,g), 512]; capsule n = 8m + 4tau + g
    wt = np.zeros((128, 16, D_IN), dtype=np.float16)
    for m in range(4):
        for g in range(4):
            for tau in range(2):
                n = 8 * m + 4 * tau + g
                wt[64 * tau:64 * tau + 64, 4 * m + g, :] = \
                    Wr[:, n, :].T.astype(np.float16)
    ident = np.eye(128, dtype=np.float16)

    in_maps = []
    for c in range(N_CORES):
        ub = u_vecs[c * B_LOC:(c + 1) * B_LOC]  # [8, 1024, 512] fp32
        u16 = ub.astype(np.float16)
        up = np.ascontiguousarray(
            u16.reshape(B_LOC, 8, 128, D_IN).transpose(0, 2, 1, 3))
        utp = np.ascontiguousarray(
            u16.transpose(0, 2, 1).reshape(B_LOC, 4, 128, N_IN)
            .transpose(0, 2, 1, 3))
        uu = np.ascontiguousarray(np.concatenate(
            [utp.reshape(B_LOC, 128, 8, D_IN), up], axis=2))
        # host iter-0: c is uniform, so outputs0 depends only on column sums
        s = ub.sum(axis=1) / NC_CAP                       # [8, 512] fp32
        pre0 = np.einsum('bk,knd->bnd', s, Wr)
        out0 = pre0 / np.sqrt((pre0 ** 2).sum(-1, keepdims=True) + EPS)
        w20 = np.einsum('bnd,knd->bnk', out0, Wr)         # [8, 32, 512]
        w20 = w20[:, np.argsort(PI), :]  # routing-domain column order pi(n)
        w20t = np.ascontiguousarray(
            w20.transpose(2, 0, 1).reshape(4, 128, B_LOC, NC_CAP)
            .transpose(1, 0, 2, 3)).astype(np.float16)
        in_maps.append({
            "uu16": uu, "w16": w16, "wt16": wt, "w20t": w20t,
            "ident": ident,
        })
    return in_maps


def _unpack_out(raw):
    # raw [128, 512] f32; row 32g+q, cols (T, d) -> out[q, 4T+g, d]
    out = np.empty((B_LOC, NC_CAP, DC), dtype=np.float32)
    r = raw.reshape(4, 32, 8, DC)   # [g, q-slot, T, d]
    for g in range(4):
        for q in range(B_LOC):
            out[q, 4 * np.arange(8) + g, :] = r[g, q]
    return out


def kernel(u_vecs, W):
    from concourse.bass_utils import run_bass_kernel_spmd

    if "nc" not in _cached:
        _cached["nc"] = _build_program()
    nc = _cached["nc"]

    in_maps = _host_prep(u_vecs, W)
    res = run_bass_kernel_spmd(nc, in_maps, list(range(N_CORES)))
    out = np.concatenate(
        [_unpack_out(res.results[c]["out"]) for c in range(N_CORES)], axis=0)
    return out.astype(np.float32)


# revision 48
# speedup vs baseline: 1.1259x; 1.0133x over previous
"""CapsNet dynamic-routing kernel for TRN2, 8 NeuronCores, data-parallel over batch.

Routing math is fully batch-local; the kernel is a per-batch pipeline hidden
under the u-vec DMA stream:

  host: iter-0 (softmax(0) is uniform) -> outputs0, w20 = W @ outputs0 shipped
  chip: per batch b:  b1 = w20 @ u^T -> softmax -> v1 = c1^T u   (as u lands)
        per group:    pre1 = v1 @ W -> squash -> w21 = W @ out1  (T-pair pipelined)
        per batch:    b2 = w21 @ u^T -> softmax -> v2
        all batches:  pre2 -> one wide squash -> single output DMA

Perf structure (measured on HW traces):
  - the DMA stream of u/u^T (16.8MB fp16) is the floor (~350-425 GB/s); the
    kernel is paced by per-batch arrival, and the tail after the last batch
    is the optimization target.  fp8 was simulated: softmax amplifies
    logit-path quantization to ~2.5e-2 rel err (budget 2e-2) - not usable.
  - u and u^T ship as ONE DMA per batch (same completion gate, fewer
    transfer boundaries); the last two batches ship u^T/u separately so the
    logits matmul can prestart during the final u transfer.
  - squash scale = exp(-0.5*ln(|pre|^2)): Ln and Exp live in the same act
    table set (natural_log_exp_and_others) => ONE act-table load total
    (Sqrt lives in a different set; Exp<->Sqrt swapping cost ~10us).
  - program order == per-engine queue order (in-order queues): every block
    is emitted in input-arrival order; nothing DMA-independent sits in
    front of DMA-dependent work.  iter-2 work packs pairwise (bup(b+1)
    between bup(b) and vmm(b)) so softmax chains hide under PE work.
  - the 4 pre pieces of a group share ONE 2KB PSUM tile (sub-tile range
    dependency tracking verified on HW), so pieces run at PE rate instead
    of serializing on a 2-buffer rotation against the squash chains.
  - iter-2 pre/squash is ONE merged 8-batch block: half the matmuls of two
    4-batch blocks, one wide squash chain, one output DMA.
  - routing-domain capsule columns (b/e/c/vT/w2T) use a permuted order
    pi(n)=8m+2g+tl for n=4(2m+tl)+g, making the w2 PSUM->SBUF gather a
    single 3D strided copy per piece instead of an 8-copy barrier.
  - the masked L copies read the transpose result straight from PSUM (one
    PSUM operand per DVE op is legal; two is not).
fp16 operands / fp32 accumulation.
"""

import numpy as np

ROUTINGS = 3
NC_CAP = 32
DC = 64
EPS = 1e-7
N_CORES = 8
B, N_IN, D_IN = 64, 1024, 512
B_LOC = B // N_CORES  # 8

# routing-domain capsule permutation: capsule n=4(2m+tl)+g lives in column
# pi(n)=8m+2g+tl of b/e/c/vT/w2T (makes the w2 gather a single strided copy)
PI = np.empty(NC_CAP, dtype=np.int64)
for _m in range(4):
    for _tl in range(2):
        for _g in range(4):
            PI[4 * (2 * _m + _tl) + _g] = 8 * _m + 2 * _g + _tl

_cached = {}


def _build_program():
    import concourse.bass as bass
    import concourse.tile as tile
    from concourse import bacc, mybir
    from concourse.hw_specs import get_activation_tables

    f16 = mybir.dt.float16
    f32 = mybir.dt.float32
    ADD = mybir.AluOpType.add
    AX = mybir.AxisListType.X
    AF = mybir.ActivationFunctionType

    nc = bacc.Bacc("TRN2", target_bir_lowering=False, debug=False,
                   num_devices=N_CORES)
    act_tabs = list(get_activation_tables(nc.m.arch).keys())
    LNEXP_SET = act_tabs.index("natural_log_exp_and_others")

    # host-packed, SBUF-native layouts (partition dim first)
    # uu16 per batch: [:, 0:8] = u^T as [128,8,512] (k-major), [:, 8:16] = u
    w16_d = nc.dram_tensor("w16", [128, 4, NC_CAP * DC], f16, kind="ExternalInput").ap()
    wt16_d = nc.dram_tensor("wt16", [128, 16, D_IN], f16, kind="ExternalInput").ap()
    w20t_d = nc.dram_tensor("w20t", [128, 4, B_LOC, NC_CAP], f16, kind="ExternalInput").ap()
    uu_d = nc.dram_tensor("uu16", [B_LOC, 128, 16, D_IN], f16, kind="ExternalInput").ap()
    ident_d = nc.dram_tensor("ident", [128, 128], f16, kind="ExternalInput").ap()
    out_d = nc.dram_tensor("out", [128, 8 * DC], f32, kind="ExternalOutput").ap()

    with tile.TileContext(nc) as tc:
        with (
            tc.tile_pool(name="big", bufs=1) as big,
            tc.tile_pool(name="work", bufs=1) as work,
            tc.tile_pool(name="sbE", bufs=2) as sbE,
            tc.tile_pool(name="sbP", bufs=2) as sbP,
            tc.tile_pool(name="sbO", bufs=4) as sbO,
            tc.tile_pool(name="psB", bufs=2, space="PSUM") as psB,
            tc.tile_pool(name="psV", bufs=1, space="PSUM") as psV,
            tc.tile_pool(name="psPre", bufs=2, space="PSUM") as psPre,
            tc.tile_pool(name="psT", bufs=2, space="PSUM") as psT,
            tc.tile_pool(name="psW2", bufs=1, space="PSUM") as psW2,
        ):
            # UU[:, b, 0:8] = u^T chunks: UT[k%128, j, i] at [:, b, 2j+i//512]
            # UU[:, b, 8:16] = u chunks:  U[i%128, t, k]  at [:, b, 8+t]
            UU = big.tile([128, B_LOC, 16, D_IN], f16, tag="UU")
            W16 = big.tile([128, 4, NC_CAP * DC], f16, tag="W16")  # (k%128),(k//128),(n d)
            WT16 = big.tile([128, 16, D_IN], f16, tag="WT16")      # (tau d),(m g),(k)
            W20T = big.tile([128, 4, B_LOC, NC_CAP], f16, tag="W20T")
            IDENT = work.tile([128, 128], f16, tag="IDENT")
            JUNK = work.tile([128, 256], f16, tag="JUNK")

            def UTv(b, j, i0):  # UT[:, b, j, i0:i0+128]
                return UU[:, b, 2 * j + i0 // 512,
                          (i0 % 512):(i0 % 512) + 128]

            def Uv(b, t, k0):   # U[:, b, t, k0:k0+128]
                return UU[:, b, 8 + t, k0:k0 + 128]

            # per-group iter-1 tiles (A: batches 0-3, B: 4-7); iter-2 shares
            vT = [work.tile([128, 4, 4, NC_CAP], f16, tag=f"vT{g}",
                            name=f"vT{g}") for g in range(2)]
            vT2 = work.tile([128, 4, B_LOC, NC_CAP], f16, tag="vT2")
            w2T = [work.tile([128, 4, 4, NC_CAP], f16, tag=f"w2T{g}",
                             name=f"w2T{g}") for g in range(2)]
            c_sb = [work.tile([128, 4, 8, NC_CAP], f16, tag=f"c{g}",
                              name=f"c{g}") for g in range(2)]
            L_sb = [[work.tile([128, 4, 2, 4], f16, tag=f"L{g}_{m}",
                               name=f"L{g}_{m}") for m in range(4)]
                    for g in range(2)]
            outp32 = work.tile([128, 8, DC], f32, tag="outp32")
            z_sb = work.tile([128, B_LOC, 8], f32, tag="z")
            r_sb = work.tile([128, B_LOC, 8], f32, tag="r")

            # ---- single act-table load: Ln+Exp (+Copy) in one set ----
            nc.scalar.add_instruction(mybir.InstLoadActFuncSet(
                name=f"I-{nc.next_id()}", act_func_set_id=LNEXP_SET))
            nc.gpsimd.memset(JUNK[:], 0.5)
            for g in range(2):
                for m in range(4):
                    nc.gpsimd.memset(L_sb[g][m][:], 0.0)

            # ---- PE HAM warm-up: ~5us of junk matmuls from t~1.5us ----
            # (shares the psW2 "w2pn" slot: w2 only starts ~40us in)
            dummy_ps = psW2.tile([128, 4, 16, 2, 4], f32, tag="w2pn")
            with nc.named_scope("ham_warmup"):
                for i in range(48):
                    nc.tensor.matmul(dummy_ps[:, i % 4], JUNK[:, 0:128],
                                     JUNK[:, 0:128],
                                     start=True, stop=True)

            def keep_warm(w):
                # junk weight-load reading a just-landed DMA slice: fires on
                # arrival, bridging a PE-idle window (pure LDWEIGHTS: no
                # PSUM write, no WAR with live accumulators)
                nc.tensor.ldweights(w)

            # ---- DMA queue: order == consumption order of the pipeline.
            # Transfers are kept at <=8KB per partition row (16KB rows
            # measured ~430ns inter-packet gaps per engine). Triggers are
            # split across the two HWDGE trigger engines (sync + scalar):
            # each trigger ring allows ~8 transfers in flight and refill
            # costs ~0.9us of dead time per transfer; two engines halve it.
            nc.sync.dma_start(IDENT[:], ident_d[:])
            nc.sync.dma_start(W20T[:], w20t_d[:])
            for b in range(0, 4):
                nc.sync.dma_start(UU[:, b, 0:8], uu_d[b][:, 0:8])
                nc.sync.dma_start(UU[:, b, 8:16], uu_d[b][:, 8:16])
            nc.sync.dma_start(W16[:, 0:2], w16_d[:, 0:2])
            nc.sync.dma_start(W16[:, 2:4], w16_d[:, 2:4])
            nc.sync.dma_start(WT16[:, 0:8], wt16_d[:, 0:8])
            nc.sync.dma_start(WT16[:, 8:16], wt16_d[:, 8:16])
            for b in range(4, 7):
                nc.sync.dma_start(UU[:, b, 0:8], uu_d[b][:, 0:8])
                nc.sync.dma_start(UU[:, b, 8:16], uu_d[b][:, 8:16])
            # the tail-gating batch ships in quarters so its logits/v
            # matmuls consume partial data as it lands (loops below
            # accumulate in matching order)
            nc.sync.dma_start(UU[:, 7, 0:4], uu_d[7][:, 0:4])
            nc.sync.dma_start(UU[:, 7, 4:8], uu_d[7][:, 4:8])
            nc.sync.dma_start(UU[:, 7, 8:12], uu_d[7][:, 8:12])
            nc.sync.dma_start(UU[:, 7, 12:16], uu_d[7][:, 12:16])

            def bup(b, it):
                # b-logits for batch b: [i%128, t, n] = sum_k u^T chunks @ w2T
                src = W20T if it == 1 else w2T[b // 4]
                bl = b if it == 1 else b % 4
                with nc.named_scope(f"i{it}_bup{b}"):
                    b_ps = psB.tile([128, 8, NC_CAP], f32, tag="b_ps")
                    for t in range(8):
                        for j in range(4):
                            nc.tensor.matmul(
                                b_ps[:, t], UTv(b, j, 128 * t),
                                src[:, j, bl, :], start=(j == 0), stop=(j == 3))
                    e_sb = sbE.tile([128, 8, NC_CAP], f16, tag="e_sb")
                    nc.scalar.activation(e_sb[:], b_ps[:], AF.Exp)
                    nc.vector.tensor_reduce(z_sb[:, b], e_sb[:], AX, ADD)
                    nc.vector.reciprocal(r_sb[:, b], z_sb[:, b])
                    nc.vector.tensor_mul(
                        c_sb[b // 4][:, b % 4], e_sb[:],
                        r_sb[:, b].broadcast_to((128, 8, NC_CAP)))

            def vmm(b, it):
                with nc.named_scope(f"i{it}_v{b}"):
                    vT_ps = psV.tile([128, 4, NC_CAP], f32, tag="vT_ps")
                    for j in range(4):
                        for t in range(8):
                            nc.tensor.matmul(
                                vT_ps[:, j], Uv(b, t, 128 * j),
                                c_sb[b // 4][:, b % 4, t, :],
                                start=(t == 0), stop=(t == 7))
                    dst = (vT[b // 4][:, :, b % 4, :] if it == 1
                           else vT2[:, :, b, :])
                    nc.scalar.copy(dst, vT_ps[:])

            def pre_halves():
                # two 1KB PSUM tiles per group (pieces 0-1 / 2-3) in
                # DIFFERENT banks: the squash front of half X runs on DVE
                # while half Y's matmuls stream (reading a PSUM bank that
                # the PE still writes serializes at bank granularity)
                pgX = psPre.tile([128, 4, DC], f32, tag="pre", name="pgX")
                pgY = psPre.tile([128, 4, DC], f32, tag="pre", name="pgY")
                return pgX, pgY

            def pre_piece(src, nb, pgh, m):
                # capsules n = 4T+g for T in {2m, 2m+1} -> pgh[:, (2m+tl)%4]
                for tl in range(2):
                    for g in range(4):
                        n = 4 * (2 * m + tl) + g
                        q = 8 * m + 2 * g + tl  # pi(n): vT column order
                        for j in range(4):
                            nc.tensor.matmul(
                                pgh[32 * g:32 * g + nb, (2 * m + tl) % 4],
                                src[:, j, :, q],
                                W16[:, j, 64 * n:64 * n + 64],
                                start=(j == 0), stop=(j == 3),
                                tile_position=(0, 32 * g),
                            )

            def squash_front(pgh, nrm, c0):
                # |pre|^2 for one half: runs while the other half's matmuls
                # stream on the PE. 1/sqrt later as exp(-0.5*ln(x)) (one act
                # table set); |pre|^2 >> 1e-7 so the reference eps is inert
                pre_c = sbP.tile([128, 4, DC], f32, tag="pre_c")
                sq2 = sbP.tile([128, 4, DC], f32, tag="sq2")
                nc.vector.tensor_copy(pre_c[:], pgh[:])
                nc.vector.tensor_mul(sq2[:], pre_c[:], pgh[:])
                nc.vector.tensor_reduce(nrm[:, c0:c0 + 4], sq2[:], AX, ADD)

            def squash_scale(nrm):
                lgn = sbP.tile([128, 8], f32, tag="lgn")
                scl = sbP.tile([128, 8], f32, tag="scl")
                nc.scalar.activation(lgn[:], nrm[:], AF.Ln)
                nc.scalar.activation(scl[:], lgn[:], AF.Exp, scale=-0.5)
                return scl

            def squash_dst(pgh, scl, c0, dst, dsl):
                nc.vector.tensor_mul(
                    dst[:, dsl], pgh[:],
                    scl[:, c0:c0 + 4].broadcast_to((128, 4, DC)))

            def w2_piece(grp, w2pn, m, o16):
                # transpose scaled outputs T-pair m -> tp[(tau d), (g c)],
                # mask into L straight from PSUM, contract d, then gather
                # with ONE strided copy (pi() makes (g,tau) a stride-4 run)
                tp = psT.tile([128, 128], f16, tag="tp")
                nc.tensor.transpose(
                    tp[:], o16[:, 2 * m:2 * m + 2].rearrange("p a b -> p (a b)"),
                    IDENT[:])
                for tau in range(2):
                    nc.vector.tensor_copy(
                        L_sb[grp][m][64 * tau:64 * tau + 64, :, tau, :],
                        tp[64 * tau:64 * tau + 64, :]
                        .rearrange("p (g c) -> p g c", g=4)[:, :, 0:4])
                for p in range(4 * m, 4 * m + 4):
                    for j in range(4):
                        nc.tensor.matmul(
                            w2pn[:, j, p], WT16[:, p, 128 * j:128 * j + 128],
                            L_sb[grp][m][:, p - 4 * m], start=True, stop=True)
                nc.scalar.copy(
                    w2T[grp][:, :, :, 8 * m:8 * m + 8],
                    w2pn[:, :, 4 * m:4 * m + 4, :, :].rearrange(
                        "p j g x b -> p j b (g x)"))

            # ================= schedule =================
            # program order == per-engine queue order; blocks emitted in the
            # order their inputs land (in-order queues => head-of-line)
            bup(0, 1); vmm(0, 1)
            bup(1, 1); vmm(1, 1)
            bup(2, 1); vmm(2, 1)
            bup(3, 1); vmm(3, 1)
            def pre1_squash_w2(grp, w2pn, mid=None):
                # `mid` (optional thunk) emits PE work between the first and
                # second pair of w2 pieces -- used to slot an arriving
                # batch's logits in so it isn't queue-blocked by the block
                pgX, pgY = pre_halves()
                nrm = sbP.tile([128, 8], f32, tag="nrm", name="nrm")
                o = sbO.tile([128, 8, DC], f16, tag="o16", name="o16")
                pre_piece(vT[grp], 4, pgX, 0)
                pre_piece(vT[grp], 4, pgX, 1)
                pre_piece(vT[grp], 4, pgY, 2)
                pre_piece(vT[grp], 4, pgY, 3)
                squash_front(pgX, nrm, 0)   # DVE, overlaps pgY matmuls
                squash_front(pgY, nrm, 4)
                scl = squash_scale(nrm)
                squash_dst(pgX, scl, 0, o, slice(0, 4))
                w2_piece(grp, w2pn, 0, o)
                w2_piece(grp, w2pn, 1, o)
                squash_dst(pgY, scl, 4, o, slice(4, 8))
                if mid is not None:
                    mid()
                w2_piece(grp, w2pn, 2, o)
                w2_piece(grp, w2pn, 3, o)

            w2pnA = psW2.tile([128, 4, 16, 2, 4], f32, tag="w2pn")
            keep_warm(W16[:, 0, 0:128])
            pre1_squash_w2(0, w2pnA, mid=lambda: bup(4, 1))
            keep_warm(WT16[:, 0, 0:128])
            # group-B iter-1 as batches land, iter-2 A packed into the
            # gaps; the final bups are pulled forward so the tail-gating
            # batch-7 logits run the moment their data lands
            bup(0, 2)
            vmm(4, 1); vmm(0, 2)
            bup(5, 1); bup(1, 2)
            vmm(5, 1); vmm(1, 2)
            bup(6, 1); bup(2, 2)
            bup(7, 1)
            vmm(6, 1); vmm(2, 2)
            bup(3, 2)
            vmm(7, 1); vmm(3, 2)
            # ---- tail: everything below is gated by the last batch ----
            w2pnB = psW2.tile([128, 4, 16, 2, 4], f32, tag="w2pn")
            pre1_squash_w2(1, w2pnB)
            # iter-2 B packed pairwise: bup(b+1) runs while softmax(b) chases
            bup(4, 2); bup(5, 2)
            vmm(4, 2); bup(6, 2)
            vmm(5, 2); bup(7, 2)
            vmm(6, 2); vmm(7, 2)
            # merged 8-batch pre2; output DMA fires per squash half
            pg2X, pg2Y = pre_halves()
            nrm2 = sbP.tile([128, 8], f32, tag="nrm", name="nrm2")
            pre_piece(vT2, 8, pg2X, 0)
            pre_piece(vT2, 8, pg2X, 1)
            pre_piece(vT2, 8, pg2Y, 2)
            pre_piece(vT2, 8, pg2Y, 3)
            squash_front(pg2X, nrm2, 0)
            squash_front(pg2Y, nrm2, 4)
            scl2 = squash_scale(nrm2)
            squash_dst(pg2X, scl2, 0, outp32, slice(0, 4))
            nc.sync.dma_start(out_d[:, 0:256],
                              outp32[:, 0:4].rearrange("p a b -> p (a b)"))
            squash_dst(pg2Y, scl2, 4, outp32, slice(4, 8))
            nc.sync.dma_start(out_d[:, 256:512],
                              outp32[:, 4:8].rearrange("p a b -> p (a b)"))

    nc.compile()
    return nc


def _host_prep(u_vecs, W):
    u_vecs = np.asarray(u_vecs, dtype=np.float32)
    W = np.asarray(W, dtype=np.float32).reshape(D_IN, NC_CAP * DC)
    Wr = W.reshape(D_IN, NC_CAP, DC)

    w16 = np.ascontiguousarray(
        W.reshape(4, 128, NC_CAP * DC).transpose(1, 0, 2)).astype(np.float16)
    # WT packed: [128=(tau,d), 16=(m,g), 512]; capsule n = 8m + 4tau + g
    wt = np.zeros((128, 16, D_IN), dtype=np.float16)
    for m in range(4):
        for g in range(4):
            for tau in range(2):
                n = 8 * m + 4 * tau + g
                wt[64 * tau:64 * tau + 64, 4 * m + g, :] = \
                    Wr[:, n, :].T.astype(np.float16)
    ident = np.eye(128, dtype=np.float16)

    in_maps = []
    for c in range(N_CORES):
        ub = u_vecs[c * B_LOC:(c + 1) * B_LOC]  # [8, 1024, 512] fp32
        u16 = ub.astype(np.float16)
        up = np.ascontiguousarray(
            u16.reshape(B_LOC, 8, 128, D_IN).transpose(0, 2, 1, 3))
        utp = np.ascontiguousarray(
            u16.transpose(0, 2, 1).reshape(B_LOC, 4, 128, N_IN)
            .transpose(0, 2, 1, 3))
        uu = np.ascontiguousarray(np.concatenate(
            [utp.reshape(B_LOC, 128, 8, D_IN), up], axis=2))
        # host iter-0: c is uniform, so outputs0 depends only on column sums
        s = ub.sum(axis=1) / NC_CAP                       # [8, 512] fp32
        pre0 = np.einsum('bk,knd->bnd', s, Wr)
        out0 = pre0 / np.sqrt((pre0 ** 2).sum(-1, keepdims=True) + EPS)
        w20 = np.einsum('bnd,knd->bnk', out0, Wr)         # [8, 32, 512]
        w20 = w20[:, np.argsort(PI), :]  # routing-domain column order pi(n)
        w20t = np.ascontiguousarray(
            w20.transpose(2, 0, 1).reshape(4, 128, B_LOC, NC_CAP)
            .transpose(1, 0, 2, 3)).astype(np.float16)
        in_maps.append({
            "uu16": uu, "w16": w16, "wt16": wt, "w20t": w20t,
            "ident": ident,
        })
    return in_maps


def _unpack_out(raw):
    # raw [128, 512] f32; row 32g+q, cols (T, d) -> out[q, 4T+g, d]
    out = np.empty((B_LOC, NC_CAP, DC), dtype=np.float32)
    r = raw.reshape(4, 32, 8, DC)   # [g, q-slot, T, d]
    for g in range(4):
        for q in range(B_LOC):
            out[q, 4 * np.arange(8) + g, :] = r[g, q]
    return out


def kernel(u_vecs, W):
    from concourse.bass_utils import run_bass_kernel_spmd

    if "nc" not in _cached:
        _cached["nc"] = _build_program()
    nc = _cached["nc"]

    in_maps = _host_prep(u_vecs, W)
    res = run_bass_kernel_spmd(nc, in_maps, list(range(N_CORES)))
    out = np.concatenate(
        [_unpack_out(res.results[c]["out"]) for c in range(N_CORES)], axis=0)
    return out.astype(np.float32)
